# revision 13
# baseline (speedup 1.0000x reference)
"""Masked multi-head attention on 8 Trainium2 NeuronCores — v2.

Reference computation (fp32):
    qkv = x @ W_qkv + b_qkv ; split q,k,v ; 16 heads, dh=64
    attn = softmax(causal(q k^T / 8)) ; z = attn v ; out = z @ W_proj + b_proj

Sharding: batch x heads. Core c owns batch c//4 and heads 4*(c%4)..4*(c%4)+4
(columns 256*(c%4):+256 of each q/k/v block of W_qkv, the matching 256 rows
of W_proj). Each core computes its 4 heads' attention for its batch and a
partial output projection [2048, 1024]; the host sums 4 partials per batch
and adds b_proj.

On-device layout highlights (per core; heads processed as 2 head-pairs):
  - x arrives pre-transposed as xt = x[b].T, streamed in [128, 8, 512]
    t-tiles so QKV starts after the first 256KB chunk arrives.
  - q^T/k^T/v^T come out of the QKV matmuls in [e, t] layout (e on
    partitions); scores are computed transposed per head-pair into a
    [128, 2, 512] PSUM tile; softmax denominators come from an
    ones-augmented AV matmul (lhsT = [v | 1]).
  - Diagonal score tiles are trapezoid-clipped: only columns [128g:512] of
    the qi window are computed/exp'd/masked/accumulated, and the causal
    mask within any clipped tile reduces to the same (col >= row) triangle,
    one on-device [128, 512] 0/1 tile built once with affine_select.
  - exp (scale=1/8 folded in) runs on Act straight out of PSUM; v bias,
    v->[t,e] repacking, and output staging run on Pool; causal-mask
    multiplies and normalization (reciprocal + PE ones-broadcast +
    multiply) run on DVE.
  - One PSUM pool of 3x[128,2,512] (6 banks) serves QKV accumulators, the
    score stream (lookahead 3), v-transposes, normalizer broadcasts, and
    projection accumulators; 2 banks hold the AV accumulators.
  - The output projection for each 512-token block is woven into the NEXT
    block's attention stream so PE fills Act's exp-latency gaps.

The harness-visible entry point is kernel(**inputs) -> np.ndarray.
"""

import sys

sys.path.insert(0, "/opt/trn_rl_repo")

import numpy as np

B = 2
S = 2048
D = 1024
NH = 16
DH = 64
NCORES = 8
TT = 512
NQI = S // TT  # 4
NKJ = S // 128  # 16


def _legalize_multi_waits(nc, max_waits=1):
    """This container's walrus rejects >1 semaphore wait per instruction
    (CoreV3 setupSyncWait "Too many sync wait commands"). Hoist extras
    onto same-engine NOPs inserted right before the offending one."""
    import concourse.mybir as mybir

    n_fixed = 0
    for fn in nc.m.functions:
        for blk in fn.blocks:
            new_insts = []
            for inst in blk.instructions:
                si = inst.sync_info
                waits = list(si.on_wait) if si is not None else []
                if len(waits) > max_waits:
                    extra, keep = waits[:-max_waits], waits[-max_waits:]
                    k = 0
                    while extra:
                        chunk, extra = extra[:max_waits], extra[max_waits:]
                        new_insts.append(
                            mybir.InstNoOp(
                                name=f"{inst.name}-wsplit{k}",
                                engine=inst.engine,
                                ins=[],
                                outs=[],
                                sync_info=mybir.SyncInfo(on_wait=chunk, on_update=[]),
                            )
                        )
                        k += 1
                    inst.sync_info = mybir.SyncInfo(
                        on_wait=keep, on_update=list(si.on_update)
                    )
                    n_fixed += 1
                new_insts.append(inst)
            blk.instructions = new_insts
    return n_fixed


def build_module(reps: int = 1, cfg: dict | None = None):
    cfg = dict(cfg or {})
    pt_bufs = cfg.get("pt_bufs", 6)
    os_bufs = cfg.get("os_bufs", 2)
    nrm_bufs = cfg.get("nrm_bufs", 2)
    n_end_cfg = cfg.get("n_end", 2)
    xt_bufs = cfg.get("xt_bufs", 3)
    psA_bufs = cfg.get("psA_bufs", 3)
    psZ_bufs = cfg.get("psZ_bufs", 2)
    msk_eng = cfg.get("msk_eng", "dve")    # mask mul: dve/pool/split
    os_eng = cfg.get("os_eng", "dve")      # out staging copies: dve/act
    pt_bf16 = cfg.get("pt_bf16", True)     # probabilities+values in bf16
    act_dma = cfg.get("act_dma", False)    # issue some DMAs from Act queue
    trap = cfg.get("trap", True)           # trapezoid-clip diagonal tiles
    weave_off = cfg.get("weave_off", False)  # disable interleaved emission
    no_ilqk = cfg.get("no_ilqk", False)    # plain (non-interleaved) tt=0 QKV
    no_defer = cfg.get("no_defer", False)  # emit v-repack before attn, not deferred
    hp0_seq = cfg.get("hp0_seq", False)    # hp0 loop without QKV weaving
    hp1_seq = cfg.get("hp1_seq", False)    # hp1 loop without proj weaving
    no_tailsplit = cfg.get("no_tailsplit", False)  # tail proj without split mms
    wmode = cfg.get("wmode", "pre_av")     # weave position: pre_av/post_av/tail
    max_weave = cfg.get("max_weave", None)  # cap woven thunks (rest sequential)
    xt_whole23 = cfg.get("xt_whole23", False)  # whole-tile loads for xt 2/3
    use_inject = cfg.get("use_inject", False)  # thread norm into next stream
    import concourse.bass as bass
    import concourse.mybir as mybir
    import concourse.tile as tile
    from concourse.bass import ts
    from concourse.masks import make_identity

    F32 = mybir.dt.float32
    F32R = mybir.dt.float32r
    BF16 = mybir.dt.bfloat16
    PT_DT = BF16 if pt_bf16 else F32R
    Identity = mybir.ActivationFunctionType.Identity
    Exp = mybir.ActivationFunctionType.Exp

    nc = bass.Bass(
        trn_type="TRN2", target_bir_lowering=False, debug=False, num_devices=NCORES
    )

    xt = nc.dram_tensor("xt", [D, S], F32R, kind="ExternalInput").ap()
    wq = nc.dram_tensor("wq", [8, 128, 256], F32R, kind="ExternalInput").ap()
    wk = nc.dram_tensor("wk", [8, 128, 256], F32R, kind="ExternalInput").ap()
    wv = nc.dram_tensor("wv", [8, 128, 256], F32R, kind="ExternalInput").ap()
    bqkv = nc.dram_tensor("bqkv", [128, 6], F32, kind="ExternalInput").ap()
    wp = nc.dram_tensor("wp", [2, 128, D], F32R, kind="ExternalInput").ap()
    out = nc.dram_tensor("out", [S, D], F32, kind="ExternalOutput").ap()

    xt_r = xt.rearrange("(o p) s -> p o s", p=128)

    with tile.TileContext(nc) as tc:
        with (
            tc.tile_pool(name="const", bufs=1) as cpool,
            tc.tile_pool(name="work", bufs=1) as wpool,
            tc.tile_pool(name="xt", bufs=xt_bufs) as xtpool,
            tc.tile_pool(name="vst", bufs=2) as vstpool,
            tc.tile_pool(name="pt", bufs=pt_bufs) as ptpool,
            tc.tile_pool(name="rec", bufs=nrm_bufs) as recpool,
            tc.tile_pool(name="rbs", bufs=nrm_bufs) as rbspool,
            tc.tile_pool(name="stg", bufs=nrm_bufs) as stgpool,
            tc.tile_pool(name="z2a", bufs=4) as z2apool,
            tc.tile_pool(name="z2b", bufs=2) as z2bpool,
            tc.tile_pool(name="os", bufs=os_bufs) as ospool,
            tc.tile_pool(name="psA", bufs=psA_bufs, space="PSUM") as psA,
            tc.tile_pool(name="psZ", bufs=psZ_bufs, space="PSUM") as psZ,
        ):
            # ---- constants (loaded/built once) ----
            # startup-critical ordering: SP streams wq per-dc (then biases,
            # wk, wp); Act streams wv per-dc then xt tile 1; the first
            # matmuls start after one wq/xt chunk each.
            wq_t = cpool.tile([128, 8, 256], F32R, tag="wq")
            wk_t = cpool.tile([128, 8, 256], F32R, tag="wk")
            wv_t = cpool.tile([128, 8, 256], F32R, tag="wv")
            bqkv_t = cpool.tile([128, 6], F32, tag="bqkv")
            for dc in range(8):
                nc.sync.dma_start(wq_t[:, dc, :], wq[dc])
            nc.sync.dma_start(bqkv_t[:], bqkv[:])
            for dc in range(8):
                nc.sync.dma_start(wk_t[:, dc, :], wk[dc])
            wv_eng = nc.scalar if act_dma else nc.sync
            for dc in range(8):
                wv_eng.dma_start(wv_t[:, dc, :], wv[dc])
            bq_t = bqkv_t[:, 0:2]
            bk_t = bqkv_t[:, 2:4]
            bv_t = bqkv_t[:, 4:6]
            wp_t = cpool.tile([128, 2, D], F32R, tag="wp")
            nc.sync.dma_start(wp_t[:], wp.rearrange("g p e -> p g e"))
            # memsets on DVE so Pool's queue opens with the xt loads; only
            # the affine_selects (gpsimd-only) sit ahead of them.
            ones_t = cpool.tile([128, 64], F32, tag="ones")
            nc.gpsimd.memset(ones_t[:], 1.0)
            ident = cpool.tile([128, 128], F32, tag="ident")
            nc.gpsimd.memset(ident[:], 1.0)
            nc.gpsimd.affine_select(
                out=ident[:],
                in_=ident[:],
                compare_op=mybir.AluOpType.is_equal,
                fill=0.0,
                base=0,
                pattern=[[-1, 128]],  # iota = row - col == 0 -> keep
                channel_multiplier=1,
            )
            # 0/1 triangle: keep (col >= row), zero elsewhere.
            msk_t = cpool.tile([128, TT], F32, tag="msk")
            nc.gpsimd.memset(msk_t[:], 1.0)
            nc.gpsimd.affine_select(
                out=msk_t[:],
                in_=msk_t[:],
                compare_op=mybir.AluOpType.is_ge,
                fill=0.0,
                base=0,
                pattern=[[1, TT]],  # iota = col - row >= 0 -> keep
                channel_multiplier=-1,
            )
            if not trap:
                # full-width per-g masks: keep col >= row + 128g
                msk4_t = cpool.tile([128, 4, TT], F32, tag="msk4")
                nc.gpsimd.memset(msk4_t[:], 1.0)
                for g in range(4):
                    nc.gpsimd.affine_select(
                        out=msk4_t[:, g, :],
                        in_=msk4_t[:, g, :],
                        compare_op=mybir.AluOpType.is_ge,
                        fill=0.0,
                        base=-128 * g,
                        pattern=[[1, TT]],
                        channel_multiplier=-1,
                    )

            os_engine = {"dve": nc.vector, "act": None, "split": "split"}[os_eng]

            def body():
                qT = wpool.tile([128, 2, S], F32R, tag="qT")
                kT = wpool.tile([128, 2, S], F32R, tag="kT")
                v_nat = wpool.tile([128, 2, NKJ, 2, 65], PT_DT, tag="v_nat")
                nc.vector.tensor_copy(
                    v_nat[:, :, :, :, 64:65],
                    ones_t[:, 0:64].rearrange(
                        "p (g a h c) -> p g a h c", g=2, a=NKJ, h=2
                    ),
                )

                xt_ts = {}

                def load_xt(tt, eng=nc.gpsimd, chunked=False):
                    t = xtpool.tile([128, 8, TT], F32R, tag="xt", name=f"xt{tt}")
                    if chunked:
                        for dc in range(8):
                            eng.dma_start(t[:, dc, :], xt_r[:, dc, ts(tt, TT)])
                    else:
                        eng.dma_start(t[:], xt_r[:, :, ts(tt, TT)])
                    xt_ts[tt] = t

                def attn(hp, qi, z2_tile, defer=None, weave=(), inject=None):
                    """Score/softmax/AV for head-pair hp over q block qi.

                    defer: emitted after the first 4 score/exp groups and
                    before their AVs (hides v-repack latency in the tt loop).
                    weave: filler thunks (QKV chunks of the next t block /
                    projection groups of the previous q block) emitted
                    BETWEEN each score/exp group and its AV so PE has
                    independent work while Act's exp drains, plus one after
                    the final AV to cover the reciprocal latency.
                    inject: the previous block's normalization closure,
                    emitted after this block's first score groups so its
                    reciprocal chain hides behind fresh PE work.
                    Returns this block's normalization closure (the caller
                    must run it, directly or via the next attn's inject).
                    """
                    n_kj = 4 * qi + 4
                    weave = list(weave)
                    wk_at = {}
                    if n_end_cfg == "adaptive":
                        ne = 2 if n_kj <= 8 else 1
                    else:
                        ne = n_end_cfg
                    n_end = min(len(weave), ne) if weave else 0
                    spread, tail_w = (
                        weave[: len(weave) - n_end],
                        weave[len(weave) - n_end :],
                    )
                    for wi, wfn in enumerate(spread):
                        wk_at.setdefault(wi * n_kj // len(spread), []).append(wfn)
                    zp = [
                        psZ.tile([65, TT], F32, tag="zp", name="zp0"),
                        psZ.tile([65, TT], F32, tag="zp", name="zp1"),
                    ]
                    pts = {}

                    def sc_exp_mask(kj):
                        g = kj - 4 * qi
                        w = TT if (g < 0 or not trap) else TT - 128 * g
                        c0 = TT - w
                        sc = psA.tile([128, 2, TT], F32, tag="big", name="sc")
                        for h in (0, 1):
                            nc.tensor.matmul(
                                sc[:, h, c0:TT],
                                kT[64 * h : 64 * h + 64, hp, ts(kj, 128)],
                                qT[64 * h : 64 * h + 64, hp, TT * qi + c0 : TT * qi + TT],
                                start=True,
                                stop=True,
                            )
                        pt = ptpool.tile([128, 2, TT], PT_DT, tag="pt", name="pt")
                        nc.scalar.activation(
                            pt[:, :, c0:TT], sc[:, :, c0:TT], Exp, scale=0.125
                        )
                        if g >= 0:
                            if msk_eng == "split":
                                mengs = (nc.vector, nc.gpsimd)
                            else:
                                mengs = (nc.vector,) * 2 if msk_eng == "dve" else (nc.gpsimd,) * 2
                            mop = msk_t[:, 0:w] if trap else msk4_t[:, g, :]
                            for h in (0, 1):
                                mengs[h].tensor_mul(
                                    pt[:, h, c0:TT], pt[:, h, c0:TT], mop
                                )
                        pts[kj] = (pt, c0)

                    def av(kj):
                        pt, c0 = pts.pop(kj)
                        for h in (0, 1):
                            nc.tensor.matmul(
                                zp[h][:, c0:TT],
                                v_nat[:, hp, kj, h, :],
                                pt[:, h, c0:TT],
                                start=(kj == 0),
                                stop=(kj == n_kj - 1),
                                skip_group_check=True,
                            )

                    if wmode == "tail":
                        tail_w = spread + tail_w
                        wk_at = {}

                    def run_weaves(kj):
                        for wfn in wk_at.get(kj, ()):
                            wfn()

                    if defer is not None:
                        for kj in range(4):
                            sc_exp_mask(kj)
                        defer()
                        if inject is not None:
                            inject()
                        for kj in range(4):
                            if wmode == "pre_av":
                                run_weaves(kj)
                                av(kj)
                            else:
                                av(kj)
                                run_weaves(kj)
                        for kj in range(4, n_kj):
                            sc_exp_mask(kj)
                            if wmode == "pre_av":
                                run_weaves(kj)
                                av(kj)
                            else:
                                av(kj)
                                run_weaves(kj)
                    else:
                        for kj in range(min(3, n_kj)):
                            sc_exp_mask(kj)
                        if inject is not None:
                            inject()
                        for kj in range(min(3, n_kj)):
                            if wmode == "pre_av":
                                run_weaves(kj)
                                av(kj)
                            else:
                                av(kj)
                                run_weaves(kj)
                        for kj in range(min(3, n_kj), n_kj):
                            sc_exp_mask(kj)
                            if wmode == "pre_av":
                                run_weaves(kj)
                                av(kj)
                            else:
                                av(kj)
                                run_weaves(kj)
                    for wfn in tail_w:
                        wfn()

                    def norm():
                        rb = psA.tile([128, 2, TT], F32, tag="big", name="rb")
                        for h in (1, 0):
                            rec = recpool.tile([65, TT], F32R, tag="rec", name="rec")
                            with nc.allow_low_precision(reason="fp32r softmax recip"):
                                nc.vector.reciprocal(rec[64:65, :], zp[h][64:65, :])
                            nc.tensor.matmul(
                                rb[0:64, h, :],
                                ones_t[64:65, 0:64].bitcast(F32R),
                                rec[64:65, :],
                                start=True,
                                stop=True,
                            )
                            rbs = rbspool.tile([64, TT], F32R, tag="rbs", name="rbs")
                            nc.vector.tensor_copy(rbs[:], rb[0:64, h, :])
                            if h == 0:
                                nc.vector.tensor_mul(
                                    z2_tile[0:64, :], zp[0][0:64, :], rbs[:]
                                )
                            else:
                                stg = stgpool.tile(
                                    [64, TT], F32R, tag="stg", name="stg"
                                )
                                nc.vector.tensor_mul(stg[:], zp[1][0:64, :], rbs[:])
                                nc.sync.dma_start(z2_tile[64:128, :], stg[:])

                    return norm

                def qkv_thunks(tt, interleave_qk=False):
                    """QKV projection for t block tt as three weavable
                    thunks (q, k, v) plus the deferred v-repack. With
                    interleave_qk (tt=0), q and k matmuls alternate per dc
                    chunk so PE consumption matches the xt DMA supply rate
                    during startup."""
                    state = {}

                    def chunk_qk():
                        xt_t = xt_ts[tt]
                        ps_q = psA.tile([128, 2, TT], F32, tag="big", name="ps_q")
                        ps_k = psA.tile([128, 2, TT], F32, tag="big", name="ps_k")
                        for dc in range(8):
                            for ps, w_t in ((ps_q, wq_t), (ps_k, wk_t)):
                                for et in (0, 1):
                                    nc.tensor.matmul(
                                        ps[:, et, :],
                                        w_t[:, dc, ts(et, 128)],
                                        xt_t[:, dc, :],
                                        start=(dc == 0),
                                        stop=(dc == 7),
                                    )
                        for dest, bias_t, ps in ((qT, bq_t, ps_q), (kT, bk_t, ps_k)):
                            for et in (0, 1):
                                nc.scalar.activation(
                                    dest[:, et, ts(tt, TT)],
                                    ps[:, et, :],
                                    Identity,
                                    bias=bias_t[:, et : et + 1],
                                )

                    def chunk(which, w_t, bias_t, dest):
                        def fn():
                            xt_t = xt_ts[tt]
                            ps = psA.tile(
                                [128, 2, TT], F32, tag="big", name=f"ps_{which}"
                            )
                            for et in (0, 1):
                                for dc in range(8):
                                    nc.tensor.matmul(
                                        ps[:, et, :],
                                        w_t[:, dc, ts(et, 128)],
                                        xt_t[:, dc, :],
                                        start=(dc == 0),
                                        stop=(dc == 7),
                                    )
                            if which != "v":
                                for et in (0, 1):
                                    nc.scalar.activation(
                                        dest[:, et, ts(tt, TT)],
                                        ps[:, et, :],
                                        Identity,
                                        bias=bias_t[:, et : et + 1],
                                    )
                            else:
                                del xt_ts[tt]
                                vsts = []
                                for et in (0, 1):
                                    vst = vstpool.tile(
                                        [128, TT], F32R, tag="vst", name="vst"
                                    )
                                    nc.scalar.activation(
                                        vst[:],
                                        ps[:, et, :],
                                        Identity,
                                        bias=bv_t[:, et : et + 1],
                                    )
                                    vsts.append(vst)
                                state["vsts"] = vsts

                        return fn

                    if interleave_qk:
                        thunks = [chunk_qk, chunk("v", wv_t, bv_t, None)]
                    else:
                        thunks = [
                            chunk("q", wq_t, bq_t, qT),
                            chunk("k", wk_t, bk_t, kT),
                            chunk("v", wv_t, bv_t, None),
                        ]

                    def vwork():
                        vneng = nc.vector
                        vsts = state["vsts"]
                        pst = psA.tile([128, 2, TT], F32, tag="big", name="pst")
                        for et in (0, 1):
                            for j4 in range(4):
                                nc.tensor.transpose(
                                    pst[:, et, ts(j4, 128)],
                                    vsts[et][:, ts(j4, 128)].bitcast(F32),
                                    ident[:],
                                )
                        for et in (0, 1):
                            for j4 in range(4):
                                vneng.tensor_copy(
                                    v_nat[:, et, 4 * tt + j4, :, 0:64],
                                    pst[:, et, ts(j4, 128)].rearrange(
                                        "p (h e) -> p h e", h=2
                                    ),
                                )

                    return thunks, vwork

                def proj_groups(qi, z2a, z2b, tail=False):
                    """One thunk per 128-token tile of the qi block. With
                    tail=True the out DMAs alternate SP/Act queues (Act is
                    idle at the end; serialized SP DMAs were the tail)."""
                    thunks = []
                    for j in range(4):
                        i = 4 * qi + j

                        def grp(j=j, i=i):
                            os_ = ospool.tile([128, D], F32, tag="os", name="os")
                            pp = psA.tile([128, 2, TT], F32, tag="big", name="pp")
                            for oh in (0, 1):
                                nc.tensor.matmul(
                                    pp[:, oh, :],
                                    z2a[:, ts(j, 128)],
                                    wp_t[:, 0, ts(oh, TT)],
                                    start=True,
                                    stop=False,
                                )
                                if tail and not no_tailsplit:
                                    # split z2b halves so the h0 rows (ready
                                    # before the h1 lane-shift DMA) project
                                    # without waiting on it
                                    nc.tensor.matmul(
                                        pp[:, oh, :],
                                        z2b[0:64, ts(j, 128)],
                                        wp_t[0:64, 1, ts(oh, TT)],
                                        start=False,
                                        stop=False,
                                    )
                                    nc.tensor.matmul(
                                        pp[:, oh, :],
                                        z2b[64:128, ts(j, 128)],
                                        wp_t[64:128, 1, ts(oh, TT)],
                                        start=False,
                                        stop=True,
                                    )
                                else:
                                    nc.tensor.matmul(
                                        pp[:, oh, :],
                                        z2b[:, ts(j, 128)],
                                        wp_t[:, 1, ts(oh, TT)],
                                        start=False,
                                        stop=True,
                                    )
                            if os_engine is None:
                                for oh in (0, 1):
                                    nc.scalar.activation(
                                        os_[:, ts(oh, TT)], pp[:, oh, :], Identity
                                    )
                            elif os_engine == "split":
                                nc.vector.tensor_copy(os_[:, 0:TT], pp[:, 0, :])
                                nc.scalar.activation(
                                    os_[:, TT:D], pp[:, 1, :], Identity
                                )
                            else:
                                os_engine.tensor_copy(os_[:], pp[:])
                            oeng = nc.scalar if act_dma else (nc.gpsimd if cfg.get("pool_odma", False) else nc.sync)
                            if tail and j == 3:
                                nc.sync.dma_start(out[ts(i, 128), 0:TT], os_[:, 0:TT])
                                oeng.dma_start(out[ts(i, 128), TT:D], os_[:, TT:D])
                            elif tail and j % 2 == 1:
                                oeng.dma_start(out[ts(i, 128), :], os_[:])
                            else:
                                nc.sync.dma_start(out[ts(i, 128), :], os_[:])

                        thunks.append(grp)
                    return thunks

                # ---- emission ----
                load_xt(0, chunked=True)
                load_xt(1, eng=(nc.scalar if act_dma else nc.gpsimd), chunked=True)
                if weave_off:
                    for tt in range(2, NQI):
                        load_xt(tt)
                    vworks = {}
                    for tt in range(NQI):
                        thunks, vworks[tt] = qkv_thunks(tt)
                        for th in thunks:
                            th()
                        vworks[tt]()
                    z2as = []
                    for tt in range(NQI):
                        z2a = z2apool.tile([128, TT], F32R, tag="z2a", name="z2a")
                        attn(0, tt, z2a)()
                        z2as.append(z2a)
                    z2bs = []
                    for qi in range(NQI):
                        z2b = z2bpool.tile([128, TT], F32R, tag="z2b", name="z2b")
                        attn(1, qi, z2b)()
                        z2bs.append(z2b)
                        for grp in proj_groups(qi, z2as[qi], z2b):
                            grp()
                    return
                thunks0, vwork0 = qkv_thunks(0, interleave_qk=not no_ilqk)
                for th in thunks0:
                    th()
                vworks = {0: vwork0}
                pending = None
                z2as = []
                for tt in range(NQI):
                    w = []
                    if tt + 1 < NQI:
                        nxt, vworks[tt + 1] = qkv_thunks(tt + 1)
                        if hp0_seq:
                            for th in nxt:
                                th()
                            if tt + 2 < NQI:
                                load_xt(tt + 2, chunked=not xt_whole23)
                        else:
                            if tt + 2 < NQI:
                                w.append(
                                    lambda tt=tt: load_xt(
                                        tt + 2, chunked=not xt_whole23
                                    )
                                )
                            w.extend(nxt)
                    z2a = z2apool.tile([128, TT], F32R, tag="z2a", name="z2a")
                    if no_defer:
                        vworks[tt]()
                        norm = attn(0, tt, z2a, weave=w, inject=pending)
                    else:
                        norm = attn(
                            0, tt, z2a, defer=vworks[tt], weave=w, inject=pending
                        )
                    if use_inject:
                        pending = norm
                    else:
                        norm()
                    z2as.append(z2a)
                z2bs = []
                for qi in range(NQI):
                    z2b = z2bpool.tile([128, TT], F32R, tag="z2b", name="z2b")
                    grps = (
                        proj_groups(qi - 1, z2as[qi - 1], z2bs[qi - 1])
                        if qi >= 1
                        else []
                    )
                    if hp1_seq:
                        nw = 0
                    elif max_weave is None:
                        nw = len(grps)
                    else:
                        nw = min(max_weave, len(grps)) if qi == 1 else 0
                    norm = attn(1, qi, z2b, weave=grps[:nw], inject=pending)
                    if use_inject:
                        pending = norm
                    else:
                        norm()
                    for grp in grps[nw:]:
                        grp()
                    z2bs.append(z2b)
                if use_inject:
                    pending()
                for grp in proj_groups(
                    NQI - 1, z2as[NQI - 1], z2bs[NQI - 1], tail=True
                ):
                    grp()

            if reps == 1:
                body()
            else:
                engs = (
                    mybir.EngineType.PE,
                    mybir.EngineType.Activation,
                    mybir.EngineType.DVE,
                    mybir.EngineType.SP,
                )
                with tc.For_i(0, reps, 1, hint_engines=engs):
                    body()

    _legalize_multi_waits(nc)
    return nc


def _host_inputs(x, W_qkv, b_qkv, W_proj):
    """Full inputs -> list of per-core input dicts."""
    x = np.asarray(x, dtype=np.float32)
    W_qkv = np.asarray(W_qkv, dtype=np.float32)
    b_qkv = np.asarray(b_qkv, dtype=np.float32)
    W_proj = np.asarray(W_proj, dtype=np.float32)

    xts = [np.ascontiguousarray(x[b].T) for b in range(B)]

    in_maps = []
    for c in range(NCORES):
        b = c // 4
        hg = c % 4
        cols = slice(256 * hg, 256 * hg + 256)
        in_maps.append(
            {
                "xt": xts[b],
                "wq": np.ascontiguousarray(W_qkv[:, 0:1024][:, cols]).reshape(
                    8, 128, 256
                ),
                "wk": np.ascontiguousarray(W_qkv[:, 1024:2048][:, cols]).reshape(
                    8, 128, 256
                ),
                "wv": np.ascontiguousarray(W_qkv[:, 2048:3072][:, cols]).reshape(
                    8, 128, 256
                ),
                "bqkv": np.concatenate(
                    [
                        b_qkv[1024 * s : 1024 * s + 1024][cols].reshape(2, 128).T
                        for s in range(3)
                    ],
                    axis=1,
                ),
                "wp": np.ascontiguousarray(W_proj[cols, :]).reshape(2, 128, D),
            }
        )
    return in_maps


_module_cache = {}

BEST_CFG = {"no_tailsplit": True, "msk_eng": "pool", "pool_odma": True, "os_bufs": 3, "n_end": 1}


def _get_module(reps: int = 1):
    if reps not in _module_cache:
        _module_cache[reps] = build_module(reps, BEST_CFG)
    return _module_cache[reps]


def run_on_device(in_maps, reps: int = 1):
    from concourse.bass_utils import run_bass_kernel_spmd

    nc = _get_module(reps)
    return run_bass_kernel_spmd(nc, in_maps, core_ids=list(range(NCORES)), trace=False)


def kernel(x, W_qkv, b_qkv, W_proj, b_proj):
    in_maps = _host_inputs(x, W_qkv, b_qkv, W_proj)
    res = run_on_device(in_maps, reps=1)
    b_proj = np.asarray(b_proj, dtype=np.float32)
    out = np.empty((B, S, D), dtype=np.float32)
    for b in range(B):
        acc = res.results[4 * b]["out"].copy()
        for c in range(4 * b + 1, 4 * b + 4):
            acc += res.results[c]["out"]
        out[b] = acc + b_proj
    return out


# revision 14
# speedup vs baseline: 1.0077x; 1.0077x over previous
"""Masked multi-head attention on 8 Trainium2 NeuronCores — v2.

Reference computation (fp32):
    qkv = x @ W_qkv + b_qkv ; split q,k,v ; 16 heads, dh=64
    attn = softmax(causal(q k^T / 8)) ; z = attn v ; out = z @ W_proj + b_proj

Sharding: batch x heads. Core c owns batch c//4 and heads 4*(c%4)..4*(c%4)+4
(columns 256*(c%4):+256 of each q/k/v block of W_qkv, the matching 256 rows
of W_proj). Each core computes its 4 heads' attention for its batch and a
partial output projection [2048, 1024]; the host sums 4 partials per batch
and adds b_proj.

On-device layout highlights (per core; heads processed as 2 head-pairs):
  - x arrives pre-transposed as xt = x[b].T, streamed in [128, 8, 512]
    t-tiles so QKV starts after the first 256KB chunk arrives.
  - q^T/k^T/v^T come out of the QKV matmuls in [e, t] layout (e on
    partitions); scores are computed transposed per head-pair into a
    [128, 2, 512] PSUM tile; softmax denominators come from an
    ones-augmented AV matmul (lhsT = [v | 1]).
  - Diagonal score tiles are trapezoid-clipped: only columns [128g:512] of
    the qi window are computed/exp'd/masked/accumulated, and the causal
    mask within any clipped tile reduces to the same (col >= row) triangle,
    one on-device [128, 512] 0/1 tile built once with affine_select.
  - exp (scale=1/8 folded in) runs on Act straight out of PSUM; v bias,
    v->[t,e] repacking, and output staging run on Pool; causal-mask
    multiplies and normalization (reciprocal + PE ones-broadcast +
    multiply) run on DVE.
  - One PSUM pool of 3x[128,2,512] (6 banks) serves QKV accumulators, the
    score stream (lookahead 3), v-transposes, normalizer broadcasts, and
    projection accumulators; 2 banks hold the AV accumulators.
  - The output projection for each 512-token block is woven into the NEXT
    block's attention stream so PE fills Act's exp-latency gaps.

The harness-visible entry point is kernel(**inputs) -> np.ndarray.
"""

import sys

sys.path.insert(0, "/opt/trn_rl_repo")

import numpy as np

B = 2
S = 2048
D = 1024
NH = 16
DH = 64
NCORES = 8
TT = 512
NQI = S // TT  # 4
NKJ = S // 128  # 16


def _legalize_multi_waits(nc, max_waits=1):
    """This container's walrus rejects >1 semaphore wait per instruction
    (CoreV3 setupSyncWait "Too many sync wait commands"). Hoist extras
    onto same-engine NOPs inserted right before the offending one."""
    import concourse.mybir as mybir

    n_fixed = 0
    for fn in nc.m.functions:
        for blk in fn.blocks:
            new_insts = []
            for inst in blk.instructions:
                si = inst.sync_info
                waits = list(si.on_wait) if si is not None else []
                if len(waits) > max_waits:
                    extra, keep = waits[:-max_waits], waits[-max_waits:]
                    k = 0
                    while extra:
                        chunk, extra = extra[:max_waits], extra[max_waits:]
                        new_insts.append(
                            mybir.InstNoOp(
                                name=f"{inst.name}-wsplit{k}",
                                engine=inst.engine,
                                ins=[],
                                outs=[],
                                sync_info=mybir.SyncInfo(on_wait=chunk, on_update=[]),
                            )
                        )
                        k += 1
                    inst.sync_info = mybir.SyncInfo(
                        on_wait=keep, on_update=list(si.on_update)
                    )
                    n_fixed += 1
                new_insts.append(inst)
            blk.instructions = new_insts
    return n_fixed


def build_module(reps: int = 1, cfg: dict | None = None):
    cfg = dict(cfg or {})
    pt_bufs = cfg.get("pt_bufs", 6)
    os_bufs = cfg.get("os_bufs", 2)
    nrm_bufs = cfg.get("nrm_bufs", 2)
    n_end_cfg = cfg.get("n_end", 2)
    xt_bufs = cfg.get("xt_bufs", 3)
    psA_bufs = cfg.get("psA_bufs", 3)
    psZ_bufs = cfg.get("psZ_bufs", 2)
    msk_eng = cfg.get("msk_eng", "dve")    # mask mul: dve/pool/split
    os_eng = cfg.get("os_eng", "dve")      # out staging copies: dve/act
    pt_bf16 = cfg.get("pt_bf16", True)     # probabilities+values in bf16
    act_dma = cfg.get("act_dma", False)    # issue some DMAs from Act queue
    trap = cfg.get("trap", True)           # trapezoid-clip diagonal tiles
    weave_off = cfg.get("weave_off", False)  # disable interleaved emission
    no_ilqk = cfg.get("no_ilqk", False)    # plain (non-interleaved) tt=0 QKV
    no_defer = cfg.get("no_defer", False)  # emit v-repack before attn, not deferred
    hp0_seq = cfg.get("hp0_seq", False)    # hp0 loop without QKV weaving
    hp1_seq = cfg.get("hp1_seq", False)    # hp1 loop without proj weaving
    no_tailsplit = cfg.get("no_tailsplit", False)  # tail proj without split mms
    wmode = cfg.get("wmode", "pre_av")     # weave position: pre_av/post_av/tail
    max_weave = cfg.get("max_weave", None)  # cap woven thunks (rest sequential)
    xt_whole23 = cfg.get("xt_whole23", False)  # whole-tile loads for xt 2/3
    use_inject = cfg.get("use_inject", False)  # thread norm into next stream
    import concourse.bass as bass
    import concourse.mybir as mybir
    import concourse.tile as tile
    from concourse.bass import ts
    from concourse.masks import make_identity

    F32 = mybir.dt.float32
    F32R = mybir.dt.float32r
    BF16 = mybir.dt.bfloat16
    PT_DT = BF16 if pt_bf16 else F32R
    Identity = mybir.ActivationFunctionType.Identity
    Exp = mybir.ActivationFunctionType.Exp

    nc = bass.Bass(
        trn_type="TRN2", target_bir_lowering=False, debug=False, num_devices=NCORES
    )

    xt = nc.dram_tensor("xt", [D, S], F32R, kind="ExternalInput").ap()
    wq = nc.dram_tensor("wq", [8, 128, 256], F32R, kind="ExternalInput").ap()
    wk = nc.dram_tensor("wk", [8, 128, 256], F32R, kind="ExternalInput").ap()
    wv = nc.dram_tensor("wv", [8, 128, 256], F32R, kind="ExternalInput").ap()
    bqkv = nc.dram_tensor("bqkv", [128, 6], F32, kind="ExternalInput").ap()
    wp = nc.dram_tensor("wp", [2, 128, D], F32R, kind="ExternalInput").ap()
    out = nc.dram_tensor("out", [S, D], F32, kind="ExternalOutput").ap()

    xt_r = xt.rearrange("(o p) s -> p o s", p=128)

    with tile.TileContext(nc) as tc:
        with (
            tc.tile_pool(name="const", bufs=1) as cpool,
            tc.tile_pool(name="work", bufs=1) as wpool,
            tc.tile_pool(name="xt", bufs=xt_bufs) as xtpool,
            tc.tile_pool(name="vst", bufs=2) as vstpool,
            tc.tile_pool(name="pt", bufs=pt_bufs) as ptpool,
            tc.tile_pool(name="rec", bufs=nrm_bufs) as recpool,
            tc.tile_pool(name="rbs", bufs=nrm_bufs) as rbspool,
            tc.tile_pool(name="stg", bufs=nrm_bufs) as stgpool,
            tc.tile_pool(name="z2a", bufs=4) as z2apool,
            tc.tile_pool(name="z2b", bufs=2) as z2bpool,
            tc.tile_pool(name="os", bufs=os_bufs) as ospool,
            tc.tile_pool(name="psA", bufs=psA_bufs, space="PSUM") as psA,
            tc.tile_pool(name="psZ", bufs=psZ_bufs, space="PSUM") as psZ,
        ):
            # ---- constants (loaded/built once) ----
            # startup-critical ordering: SP streams wq per-dc (then biases,
            # wk, wp); Act streams wv per-dc then xt tile 1; the first
            # matmuls start after one wq/xt chunk each.
            wq_t = cpool.tile([128, 8, 256], F32R, tag="wq")
            wk_t = cpool.tile([128, 8, 256], F32R, tag="wk")
            wv_t = cpool.tile([128, 8, 256], F32R, tag="wv")
            bqkv_t = cpool.tile([128, 6], F32, tag="bqkv")
            for dc in range(8):
                nc.sync.dma_start(wq_t[:, dc, :], wq[dc])
            nc.sync.dma_start(bqkv_t[:], bqkv[:])
            for dc in range(8):
                nc.sync.dma_start(wk_t[:, dc, :], wk[dc])
            wv_eng = nc.scalar if act_dma else nc.sync
            for dc in range(8):
                wv_eng.dma_start(wv_t[:, dc, :], wv[dc])
            bq_t = bqkv_t[:, 0:2]
            bk_t = bqkv_t[:, 2:4]
            bv_t = bqkv_t[:, 4:6]
            wp_t = cpool.tile([128, 2, D], F32R, tag="wp")
            nc.sync.dma_start(wp_t[:], wp.rearrange("g p e -> p g e"))
            # memsets on DVE so Pool's queue opens with the xt loads; only
            # the affine_selects (gpsimd-only) sit ahead of them.
            ones_t = cpool.tile([128, 64], F32, tag="ones")
            nc.gpsimd.memset(ones_t[:], 1.0)
            ident = cpool.tile([128, 128], F32, tag="ident")
            nc.gpsimd.memset(ident[:], 1.0)
            nc.gpsimd.affine_select(
                out=ident[:],
                in_=ident[:],
                compare_op=mybir.AluOpType.is_equal,
                fill=0.0,
                base=0,
                pattern=[[-1, 128]],  # iota = row - col == 0 -> keep
                channel_multiplier=1,
            )
            ident16 = cpool.tile([128, 128], PT_DT, tag="ident16")
            nc.vector.tensor_copy(ident16[:], ident[:])
            # 0/1 triangle: keep (col >= row), zero elsewhere.
            msk_t = cpool.tile([128, TT], F32, tag="msk")
            nc.gpsimd.memset(msk_t[:], 1.0)
            nc.gpsimd.affine_select(
                out=msk_t[:],
                in_=msk_t[:],
                compare_op=mybir.AluOpType.is_ge,
                fill=0.0,
                base=0,
                pattern=[[1, TT]],  # iota = col - row >= 0 -> keep
                channel_multiplier=-1,
            )
            if not trap:
                # full-width per-g masks: keep col >= row + 128g
                msk4_t = cpool.tile([128, 4, TT], F32, tag="msk4")
                nc.gpsimd.memset(msk4_t[:], 1.0)
                for g in range(4):
                    nc.gpsimd.affine_select(
                        out=msk4_t[:, g, :],
                        in_=msk4_t[:, g, :],
                        compare_op=mybir.AluOpType.is_ge,
                        fill=0.0,
                        base=-128 * g,
                        pattern=[[1, TT]],
                        channel_multiplier=-1,
                    )

            os_engine = {"dve": nc.vector, "act": None, "split": "split"}[os_eng]

            def body():
                qT = wpool.tile([128, 2, S], F32R, tag="qT")
                kT = wpool.tile([128, 2, S], F32R, tag="kT")
                v_nat = wpool.tile([128, 2, NKJ, 2, 65], PT_DT, tag="v_nat")
                nc.vector.tensor_copy(
                    v_nat[:, :, :, :, 64:65],
                    ones_t[:, 0:64].rearrange(
                        "p (g a h c) -> p g a h c", g=2, a=NKJ, h=2
                    ),
                )

                xt_ts = {}

                def load_xt(tt, eng=nc.gpsimd, chunked=False):
                    t = xtpool.tile([128, 8, TT], F32R, tag="xt", name=f"xt{tt}")
                    if chunked:
                        for dc in range(8):
                            eng.dma_start(t[:, dc, :], xt_r[:, dc, ts(tt, TT)])
                    else:
                        eng.dma_start(t[:], xt_r[:, :, ts(tt, TT)])
                    xt_ts[tt] = t

                def attn(hp, qi, z2_tile, defer=None, weave=(), inject=None):
                    """Score/softmax/AV for head-pair hp over q block qi.

                    defer: emitted after the first 4 score/exp groups and
                    before their AVs (hides v-repack latency in the tt loop).
                    weave: filler thunks (QKV chunks of the next t block /
                    projection groups of the previous q block) emitted
                    BETWEEN each score/exp group and its AV so PE has
                    independent work while Act's exp drains, plus one after
                    the final AV to cover the reciprocal latency.
                    inject: the previous block's normalization closure,
                    emitted after this block's first score groups so its
                    reciprocal chain hides behind fresh PE work.
                    Returns this block's normalization closure (the caller
                    must run it, directly or via the next attn's inject).
                    """
                    n_kj = 4 * qi + 4
                    weave = list(weave)
                    wk_at = {}
                    if n_end_cfg == "adaptive":
                        ne = 2 if n_kj <= 8 else 1
                    else:
                        ne = n_end_cfg
                    n_end = min(len(weave), ne) if weave else 0
                    spread, tail_w = (
                        weave[: len(weave) - n_end],
                        weave[len(weave) - n_end :],
                    )
                    for wi, wfn in enumerate(spread):
                        wk_at.setdefault(wi * n_kj // len(spread), []).append(wfn)
                    zp = [
                        psZ.tile([65, TT], F32, tag="zp", name="zp0"),
                        psZ.tile([65, TT], F32, tag="zp", name="zp1"),
                    ]
                    pts = {}

                    def sc_exp_mask(kj):
                        g = kj - 4 * qi
                        w = TT if (g < 0 or not trap) else TT - 128 * g
                        c0 = TT - w
                        sc = psA.tile([128, 2, TT], F32, tag="big", name="sc")
                        for h in (0, 1):
                            nc.tensor.matmul(
                                sc[:, h, c0:TT],
                                kT[64 * h : 64 * h + 64, hp, ts(kj, 128)],
                                qT[64 * h : 64 * h + 64, hp, TT * qi + c0 : TT * qi + TT],
                                start=True,
                                stop=True,
                            )
                        pt = ptpool.tile([128, 2, TT], PT_DT, tag="pt", name="pt")
                        nc.scalar.activation(
                            pt[:, :, c0:TT], sc[:, :, c0:TT], Exp, scale=0.125
                        )
                        if g >= 0:
                            if msk_eng == "split":
                                mengs = (nc.vector, nc.gpsimd)
                            else:
                                mengs = (nc.vector,) * 2 if msk_eng == "dve" else (nc.gpsimd,) * 2
                            mop = msk_t[:, 0:w] if trap else msk4_t[:, g, :]
                            for h in (0, 1):
                                mengs[h].tensor_mul(
                                    pt[:, h, c0:TT], pt[:, h, c0:TT], mop
                                )
                        pts[kj] = (pt, c0)

                    def av(kj):
                        pt, c0 = pts.pop(kj)
                        for h in (0, 1):
                            nc.tensor.matmul(
                                zp[h][:, c0:TT],
                                v_nat[:, hp, kj, h, :],
                                pt[:, h, c0:TT],
                                start=(kj == 0),
                                stop=(kj == n_kj - 1),
                                skip_group_check=True,
                            )

                    if wmode == "tail":
                        tail_w = spread + tail_w
                        wk_at = {}

                    def run_weaves(kj):
                        for wfn in wk_at.get(kj, ()):
                            wfn()

                    if defer is not None:
                        for kj in range(4):
                            sc_exp_mask(kj)
                        defer()
                        if inject is not None:
                            inject()
                        for kj in range(4):
                            if wmode == "pre_av":
                                run_weaves(kj)
                                av(kj)
                            else:
                                av(kj)
                                run_weaves(kj)
                        for kj in range(4, n_kj):
                            sc_exp_mask(kj)
                            if wmode == "pre_av":
                                run_weaves(kj)
                                av(kj)
                            else:
                                av(kj)
                                run_weaves(kj)
                    else:
                        for kj in range(min(3, n_kj)):
                            sc_exp_mask(kj)
                        if inject is not None:
                            inject()
                        for kj in range(min(3, n_kj)):
                            if wmode == "pre_av":
                                run_weaves(kj)
                                av(kj)
                            else:
                                av(kj)
                                run_weaves(kj)
                        for kj in range(min(3, n_kj), n_kj):
                            sc_exp_mask(kj)
                            if wmode == "pre_av":
                                run_weaves(kj)
                                av(kj)
                            else:
                                av(kj)
                                run_weaves(kj)
                    for wfn in tail_w:
                        wfn()

                    def norm():
                        rb = psA.tile([128, 2, TT], F32, tag="big", name="rb")
                        for h in (1, 0):
                            rec = recpool.tile([65, TT], F32R, tag="rec", name="rec")
                            with nc.allow_low_precision(reason="fp32r softmax recip"):
                                nc.vector.reciprocal(rec[64:65, :], zp[h][64:65, :])
                            nc.tensor.matmul(
                                rb[0:64, h, :],
                                ones_t[64:65, 0:64].bitcast(F32R),
                                rec[64:65, :],
                                start=True,
                                stop=True,
                            )
                            rbs = rbspool.tile([64, TT], F32R, tag="rbs", name="rbs")
                            nc.vector.tensor_copy(rbs[:], rb[0:64, h, :])
                            if h == 0:
                                nc.vector.tensor_mul(
                                    z2_tile[0:64, :], zp[0][0:64, :], rbs[:]
                                )
                            else:
                                stg = stgpool.tile(
                                    [64, TT], F32R, tag="stg", name="stg"
                                )
                                nc.vector.tensor_mul(stg[:], zp[1][0:64, :], rbs[:])
                                nc.sync.dma_start(z2_tile[64:128, :], stg[:])

                    return norm

                def qkv_thunks(tt, interleave_qk=False):
                    """QKV projection for t block tt as three weavable
                    thunks (q, k, v) plus the deferred v-repack. With
                    interleave_qk (tt=0), q and k matmuls alternate per dc
                    chunk so PE consumption matches the xt DMA supply rate
                    during startup."""
                    state = {}

                    def chunk_qk():
                        xt_t = xt_ts[tt]
                        ps_q = psA.tile([128, 2, TT], F32, tag="big", name="ps_q")
                        ps_k = psA.tile([128, 2, TT], F32, tag="big", name="ps_k")
                        for dc in range(8):
                            for ps, w_t in ((ps_q, wq_t), (ps_k, wk_t)):
                                for et in (0, 1):
                                    nc.tensor.matmul(
                                        ps[:, et, :],
                                        w_t[:, dc, ts(et, 128)],
                                        xt_t[:, dc, :],
                                        start=(dc == 0),
                                        stop=(dc == 7),
                                    )
                        for dest, bias_t, ps in ((qT, bq_t, ps_q), (kT, bk_t, ps_k)):
                            for et in (0, 1):
                                nc.scalar.activation(
                                    dest[:, et, ts(tt, TT)],
                                    ps[:, et, :],
                                    Identity,
                                    bias=bias_t[:, et : et + 1],
                                )

                    def chunk(which, w_t, bias_t, dest):
                        def fn():
                            xt_t = xt_ts[tt]
                            ps = psA.tile(
                                [128, 2, TT], F32, tag="big", name=f"ps_{which}"
                            )
                            for et in (0, 1):
                                for dc in range(8):
                                    nc.tensor.matmul(
                                        ps[:, et, :],
                                        w_t[:, dc, ts(et, 128)],
                                        xt_t[:, dc, :],
                                        start=(dc == 0),
                                        stop=(dc == 7),
                                    )
                            if which != "v":
                                for et in (0, 1):
                                    nc.scalar.activation(
                                        dest[:, et, ts(tt, TT)],
                                        ps[:, et, :],
                                        Identity,
                                        bias=bias_t[:, et : et + 1],
                                    )
                            else:
                                del xt_ts[tt]
                                vsts = []
                                for et in (0, 1):
                                    vst = vstpool.tile(
                                        [128, TT], PT_DT, tag="vst", name="vst"
                                    )
                                    nc.scalar.activation(
                                        vst[:],
                                        ps[:, et, :],
                                        Identity,
                                        bias=bv_t[:, et : et + 1],
                                    )
                                    vsts.append(vst)
                                state["vsts"] = vsts

                        return fn

                    if interleave_qk:
                        thunks = [chunk_qk, chunk("v", wv_t, bv_t, None)]
                    else:
                        thunks = [
                            chunk("q", wq_t, bq_t, qT),
                            chunk("k", wk_t, bk_t, kT),
                            chunk("v", wv_t, bv_t, None),
                        ]

                    def vwork():
                        vneng = nc.vector
                        vsts = state["vsts"]
                        pst = psA.tile([128, 2, TT], PT_DT, tag="big", name="pst")
                        for et in (0, 1):
                            for j4 in range(4):
                                nc.tensor.transpose(
                                    pst[:, et, ts(j4, 128)],
                                    vsts[et][:, ts(j4, 128)],
                                    ident16[:],
                                )
                        for et in (0, 1):
                            for j4 in range(4):
                                vneng.tensor_copy(
                                    v_nat[:, et, 4 * tt + j4, :, 0:64],
                                    pst[:, et, ts(j4, 128)].rearrange(
                                        "p (h e) -> p h e", h=2
                                    ),
                                )

                    return thunks, vwork

                def proj_groups(qi, z2a, z2b, tail=False):
                    """One thunk per 128-token tile of the qi block. With
                    tail=True the out DMAs alternate SP/Act queues (Act is
                    idle at the end; serialized SP DMAs were the tail)."""
                    thunks = []
                    for j in range(4):
                        i = 4 * qi + j

                        def grp(j=j, i=i):
                            os_ = ospool.tile([128, D], F32, tag="os", name="os")
                            pp = psA.tile([128, 2, TT], F32, tag="big", name="pp")
                            for oh in (0, 1):
                                nc.tensor.matmul(
                                    pp[:, oh, :],
                                    z2a[:, ts(j, 128)],
                                    wp_t[:, 0, ts(oh, TT)],
                                    start=True,
                                    stop=False,
                                )
                                if tail and not no_tailsplit:
                                    # split z2b halves so the h0 rows (ready
                                    # before the h1 lane-shift DMA) project
                                    # without waiting on it
                                    nc.tensor.matmul(
                                        pp[:, oh, :],
                                        z2b[0:64, ts(j, 128)],
                                        wp_t[0:64, 1, ts(oh, TT)],
                                        start=False,
                                        stop=False,
                                    )
                                    nc.tensor.matmul(
                                        pp[:, oh, :],
                                        z2b[64:128, ts(j, 128)],
                                        wp_t[64:128, 1, ts(oh, TT)],
                                        start=False,
                                        stop=True,
                                    )
                                else:
                                    nc.tensor.matmul(
                                        pp[:, oh, :],
                                        z2b[:, ts(j, 128)],
                                        wp_t[:, 1, ts(oh, TT)],
                                        start=False,
                                        stop=True,
                                    )
                            if os_engine is None:
                                for oh in (0, 1):
                                    nc.scalar.activation(
                                        os_[:, ts(oh, TT)], pp[:, oh, :], Identity
                                    )
                            elif os_engine == "split":
                                nc.vector.tensor_copy(os_[:, 0:TT], pp[:, 0, :])
                                nc.scalar.activation(
                                    os_[:, TT:D], pp[:, 1, :], Identity
                                )
                            else:
                                os_engine.tensor_copy(os_[:], pp[:])
                            oeng = nc.scalar if act_dma else (nc.gpsimd if cfg.get("pool_odma", False) else nc.sync)
                            if tail and j == 3:
                                nc.sync.dma_start(out[ts(i, 128), 0:TT], os_[:, 0:TT])
                                oeng.dma_start(out[ts(i, 128), TT:D], os_[:, TT:D])
                            elif tail and j % 2 == 1:
                                oeng.dma_start(out[ts(i, 128), :], os_[:])
                            else:
                                nc.sync.dma_start(out[ts(i, 128), :], os_[:])

                        thunks.append(grp)
                    return thunks

                # ---- emission ----
                load_xt(0, chunked=True)
                load_xt(1, eng=(nc.scalar if act_dma else nc.gpsimd), chunked=True)
                if weave_off:
                    for tt in range(2, NQI):
                        load_xt(tt)
                    vworks = {}
                    for tt in range(NQI):
                        thunks, vworks[tt] = qkv_thunks(tt)
                        for th in thunks:
                            th()
                        vworks[tt]()
                    z2as = []
                    for tt in range(NQI):
                        z2a = z2apool.tile([128, TT], F32R, tag="z2a", name="z2a")
                        attn(0, tt, z2a)()
                        z2as.append(z2a)
                    z2bs = []
                    for qi in range(NQI):
                        z2b = z2bpool.tile([128, TT], F32R, tag="z2b", name="z2b")
                        attn(1, qi, z2b)()
                        z2bs.append(z2b)
                        for grp in proj_groups(qi, z2as[qi], z2b):
                            grp()
                    return
                thunks0, vwork0 = qkv_thunks(0, interleave_qk=not no_ilqk)
                for th in thunks0:
                    th()
                vworks = {0: vwork0}
                pending = None
                z2as = []
                for tt in range(NQI):
                    w = []
                    if tt + 1 < NQI:
                        nxt, vworks[tt + 1] = qkv_thunks(tt + 1)
                        if hp0_seq:
                            for th in nxt:
                                th()
                            if tt + 2 < NQI:
                                load_xt(tt + 2, chunked=not xt_whole23)
                        else:
                            if tt + 2 < NQI:
                                w.append(
                                    lambda tt=tt: load_xt(
                                        tt + 2, chunked=not xt_whole23
                                    )
                                )
                            w.extend(nxt)
                    z2a = z2apool.tile([128, TT], F32R, tag="z2a", name="z2a")
                    if no_defer:
                        vworks[tt]()
                        norm = attn(0, tt, z2a, weave=w, inject=pending)
                    else:
                        norm = attn(
                            0, tt, z2a, defer=vworks[tt], weave=w, inject=pending
                        )
                    if use_inject:
                        pending = norm
                    else:
                        norm()
                    z2as.append(z2a)
                z2bs = []
                for qi in range(NQI):
                    z2b = z2bpool.tile([128, TT], F32R, tag="z2b", name="z2b")
                    grps = (
                        proj_groups(qi - 1, z2as[qi - 1], z2bs[qi - 1])
                        if qi >= 1
                        else []
                    )
                    if hp1_seq:
                        nw = 0
                    elif max_weave is None:
                        nw = len(grps)
                    else:
                        nw = min(max_weave, len(grps)) if qi == 1 else 0
                    norm = attn(1, qi, z2b, weave=grps[:nw], inject=pending)
                    if use_inject:
                        pending = norm
                    else:
                        norm()
                    for grp in grps[nw:]:
                        grp()
                    z2bs.append(z2b)
                if use_inject:
                    pending()
                for grp in proj_groups(
                    NQI - 1, z2as[NQI - 1], z2bs[NQI - 1], tail=True
                ):
                    grp()

            if reps == 1:
                body()
            else:
                engs = (
                    mybir.EngineType.PE,
                    mybir.EngineType.Activation,
                    mybir.EngineType.DVE,
                    mybir.EngineType.SP,
                )
                with tc.For_i(0, reps, 1, hint_engines=engs):
                    body()

    _legalize_multi_waits(nc)
    return nc


def _host_inputs(x, W_qkv, b_qkv, W_proj):
    """Full inputs -> list of per-core input dicts."""
    x = np.asarray(x, dtype=np.float32)
    W_qkv = np.asarray(W_qkv, dtype=np.float32)
    b_qkv = np.asarray(b_qkv, dtype=np.float32)
    W_proj = np.asarray(W_proj, dtype=np.float32)

    xts = [np.ascontiguousarray(x[b].T) for b in range(B)]

    in_maps = []
    for c in range(NCORES):
        b = c // 4
        hg = c % 4
        cols = slice(256 * hg, 256 * hg + 256)
        in_maps.append(
            {
                "xt": xts[b],
                "wq": np.ascontiguousarray(W_qkv[:, 0:1024][:, cols]).reshape(
                    8, 128, 256
                ),
                "wk": np.ascontiguousarray(W_qkv[:, 1024:2048][:, cols]).reshape(
                    8, 128, 256
                ),
                "wv": np.ascontiguousarray(W_qkv[:, 2048:3072][:, cols]).reshape(
                    8, 128, 256
                ),
                "bqkv": np.concatenate(
                    [
                        b_qkv[1024 * s : 1024 * s + 1024][cols].reshape(2, 128).T
                        for s in range(3)
                    ],
                    axis=1,
                ),
                "wp": np.ascontiguousarray(W_proj[cols, :]).reshape(2, 128, D),
            }
        )
    return in_maps


_module_cache = {}

BEST_CFG = {"no_tailsplit": True, "msk_eng": "pool", "pool_odma": True, "os_bufs": 3, "n_end": 1}


def _get_module(reps: int = 1):
    if reps not in _module_cache:
        _module_cache[reps] = build_module(reps, BEST_CFG)
    return _module_cache[reps]


def run_on_device(in_maps, reps: int = 1):
    from concourse.bass_utils import run_bass_kernel_spmd

    nc = _get_module(reps)
    return run_bass_kernel_spmd(nc, in_maps, core_ids=list(range(NCORES)), trace=False)


def kernel(x, W_qkv, b_qkv, W_proj, b_proj):
    in_maps = _host_inputs(x, W_qkv, b_qkv, W_proj)
    res = run_on_device(in_maps, reps=1)
    b_proj = np.asarray(b_proj, dtype=np.float32)
    out = np.empty((B, S, D), dtype=np.float32)
    for b in range(B):
        acc = res.results[4 * b]["out"].copy()
        for c in range(4 * b + 1, 4 * b + 4):
            acc += res.results[c]["out"]
        out[b] = acc + b_proj
    return out


# revision 15
# speedup vs baseline: 1.0161x; 1.0083x over previous
"""Masked multi-head attention on 8 Trainium2 NeuronCores — v2.

Reference computation (fp32):
    qkv = x @ W_qkv + b_qkv ; split q,k,v ; 16 heads, dh=64
    attn = softmax(causal(q k^T / 8)) ; z = attn v ; out = z @ W_proj + b_proj

Sharding: batch x heads. Core c owns batch c//4 and heads 4*(c%4)..4*(c%4)+4
(columns 256*(c%4):+256 of each q/k/v block of W_qkv, the matching 256 rows
of W_proj). Each core computes its 4 heads' attention for its batch and a
partial output projection [2048, 1024]; the host sums 4 partials per batch
and adds b_proj.

On-device layout highlights (per core; heads processed as 2 head-pairs):
  - x arrives pre-transposed as xt = x[b].T, streamed in [128, 8, 512]
    t-tiles so QKV starts after the first 256KB chunk arrives.
  - q^T/k^T/v^T come out of the QKV matmuls in [e, t] layout (e on
    partitions); scores are computed transposed per head-pair into a
    [128, 2, 512] PSUM tile; softmax denominators come from an
    ones-augmented AV matmul (lhsT = [v | 1]).
  - Diagonal score tiles are trapezoid-clipped: only columns [128g:512] of
    the qi window are computed/exp'd/masked/accumulated, and the causal
    mask within any clipped tile reduces to the same (col >= row) triangle,
    one on-device [128, 512] 0/1 tile built once with affine_select.
  - exp (scale=1/8 folded in) runs on Act straight out of PSUM; v bias,
    v->[t,e] repacking, and output staging run on Pool; causal-mask
    multiplies and normalization (reciprocal + PE ones-broadcast +
    multiply) run on DVE.
  - One PSUM pool of 3x[128,2,512] (6 banks) serves QKV accumulators, the
    score stream (lookahead 3), v-transposes, normalizer broadcasts, and
    projection accumulators; 2 banks hold the AV accumulators.
  - The output projection for each 512-token block is woven into the NEXT
    block's attention stream so PE fills Act's exp-latency gaps.

The harness-visible entry point is kernel(**inputs) -> np.ndarray.
"""

import sys

sys.path.insert(0, "/opt/trn_rl_repo")

import numpy as np

B = 2
S = 2048
D = 1024
NH = 16
DH = 64
NCORES = 8
TT = 512
NQI = S // TT  # 4
NKJ = S // 128  # 16


def _legalize_multi_waits(nc, max_waits=1):
    """This container's walrus rejects >1 semaphore wait per instruction
    (CoreV3 setupSyncWait "Too many sync wait commands"). Hoist extras
    onto same-engine NOPs inserted right before the offending one."""
    import concourse.mybir as mybir

    n_fixed = 0
    for fn in nc.m.functions:
        for blk in fn.blocks:
            new_insts = []
            for inst in blk.instructions:
                si = inst.sync_info
                waits = list(si.on_wait) if si is not None else []
                if len(waits) > max_waits:
                    extra, keep = waits[:-max_waits], waits[-max_waits:]
                    k = 0
                    while extra:
                        chunk, extra = extra[:max_waits], extra[max_waits:]
                        new_insts.append(
                            mybir.InstNoOp(
                                name=f"{inst.name}-wsplit{k}",
                                engine=inst.engine,
                                ins=[],
                                outs=[],
                                sync_info=mybir.SyncInfo(on_wait=chunk, on_update=[]),
                            )
                        )
                        k += 1
                    inst.sync_info = mybir.SyncInfo(
                        on_wait=keep, on_update=list(si.on_update)
                    )
                    n_fixed += 1
                new_insts.append(inst)
            blk.instructions = new_insts
    return n_fixed


def build_module(reps: int = 1, cfg: dict | None = None):
    cfg = dict(cfg or {})
    pt_bufs = cfg.get("pt_bufs", 6)
    os_bufs = cfg.get("os_bufs", 2)
    nrm_bufs = cfg.get("nrm_bufs", 2)
    n_end_cfg = cfg.get("n_end", 2)
    xt_bufs = cfg.get("xt_bufs", 3)
    psA_bufs = cfg.get("psA_bufs", 3)
    psZ_bufs = cfg.get("psZ_bufs", 2)
    msk_eng = cfg.get("msk_eng", "dve")    # mask mul: dve/pool/split
    os_eng = cfg.get("os_eng", "dve")      # out staging copies: dve/act
    pt_bf16 = cfg.get("pt_bf16", True)     # probabilities+values in bf16
    act_dma = cfg.get("act_dma", False)    # issue some DMAs from Act queue
    trap = cfg.get("trap", True)           # trapezoid-clip diagonal tiles
    weave_off = cfg.get("weave_off", False)  # disable interleaved emission
    no_ilqk = cfg.get("no_ilqk", False)    # plain (non-interleaved) tt=0 QKV
    no_defer = cfg.get("no_defer", False)  # emit v-repack before attn, not deferred
    hp0_seq = cfg.get("hp0_seq", False)    # hp0 loop without QKV weaving
    hp1_seq = cfg.get("hp1_seq", False)    # hp1 loop without proj weaving
    no_tailsplit = cfg.get("no_tailsplit", False)  # tail proj without split mms
    wmode = cfg.get("wmode", "pre_av")     # weave position: pre_av/post_av/tail
    max_weave = cfg.get("max_weave", None)  # cap woven thunks (rest sequential)
    xt_whole23 = cfg.get("xt_whole23", False)  # whole-tile loads for xt 2/3
    use_inject = cfg.get("use_inject", False)  # thread norm into next stream
    import concourse.bass as bass
    import concourse.mybir as mybir
    import concourse.tile as tile
    from concourse.bass import ts
    from concourse.masks import make_identity

    F32 = mybir.dt.float32
    F32R = mybir.dt.float32r
    BF16 = mybir.dt.bfloat16
    PT_DT = BF16 if pt_bf16 else F32R
    Identity = mybir.ActivationFunctionType.Identity
    Exp = mybir.ActivationFunctionType.Exp

    nc = bass.Bass(
        trn_type="TRN2", target_bir_lowering=False, debug=False, num_devices=NCORES
    )

    xt = nc.dram_tensor("xt", [D, S], F32R, kind="ExternalInput").ap()
    wq = nc.dram_tensor("wq", [8, 128, 256], F32R, kind="ExternalInput").ap()
    wk = nc.dram_tensor("wk", [8, 128, 256], F32R, kind="ExternalInput").ap()
    wv = nc.dram_tensor("wv", [8, 128, 256], F32R, kind="ExternalInput").ap()
    bqkv = nc.dram_tensor("bqkv", [128, 6], F32, kind="ExternalInput").ap()
    wp = nc.dram_tensor("wp", [2, 128, D], F32R, kind="ExternalInput").ap()
    out = nc.dram_tensor("out", [S, D], F32, kind="ExternalOutput").ap()

    xt_r = xt.rearrange("(o p) s -> p o s", p=128)

    with tile.TileContext(nc) as tc:
        with (
            tc.tile_pool(name="const", bufs=1) as cpool,
            tc.tile_pool(name="work", bufs=1) as wpool,
            tc.tile_pool(name="xt", bufs=xt_bufs) as xtpool,
            tc.tile_pool(name="vst", bufs=2) as vstpool,
            tc.tile_pool(name="pt", bufs=pt_bufs) as ptpool,
            tc.tile_pool(name="rec", bufs=nrm_bufs) as recpool,
            tc.tile_pool(name="rbs", bufs=nrm_bufs) as rbspool,
            tc.tile_pool(name="stg", bufs=nrm_bufs) as stgpool,
            tc.tile_pool(name="z2a", bufs=4) as z2apool,
            tc.tile_pool(name="z2b", bufs=2) as z2bpool,
            tc.tile_pool(name="os", bufs=os_bufs) as ospool,
            tc.tile_pool(name="psA", bufs=psA_bufs, space="PSUM") as psA,
            tc.tile_pool(name="psZ", bufs=psZ_bufs, space="PSUM") as psZ,
        ):
            # ---- constants (loaded/built once) ----
            # startup-critical ordering: SP streams wq per-dc (then biases,
            # wk, wp); Act streams wv per-dc then xt tile 1; the first
            # matmuls start after one wq/xt chunk each.
            wq_t = cpool.tile([128, 8, 256], F32R, tag="wq")
            wk_t = cpool.tile([128, 8, 256], F32R, tag="wk")
            wv_t = cpool.tile([128, 8, 256], F32R, tag="wv")
            bqkv_t = cpool.tile([128, 6], F32, tag="bqkv")
            for dc in range(8):
                nc.sync.dma_start(wq_t[:, dc, :], wq[dc])
            nc.sync.dma_start(bqkv_t[:], bqkv[:])
            for dc in range(8):
                nc.sync.dma_start(wk_t[:, dc, :], wk[dc])
            wv_eng = nc.scalar if act_dma else nc.sync
            for dc in range(8):
                wv_eng.dma_start(wv_t[:, dc, :], wv[dc])
            bq_t = bqkv_t[:, 0:2]
            bk_t = bqkv_t[:, 2:4]
            bv_t = bqkv_t[:, 4:6]
            wp_t = cpool.tile([128, 2, D], F32R, tag="wp")
            nc.sync.dma_start(wp_t[:], wp.rearrange("g p e -> p g e"))
            # memsets on DVE so Pool's queue opens with the xt loads; only
            # the affine_selects (gpsimd-only) sit ahead of them.
            ones_t = cpool.tile([128, 64], F32, tag="ones")
            ident = cpool.tile([128, 128], F32, tag="ident")
            ident16 = cpool.tile([128, 128], PT_DT, tag="ident16")
            msk_t = cpool.tile([128, TT], F32, tag="msk")

            def build_consts():
                nc.gpsimd.memset(ones_t[:], 1.0)
                nc.gpsimd.memset(ident[:], 1.0)
                nc.gpsimd.affine_select(
                    out=ident[:],
                    in_=ident[:],
                    compare_op=mybir.AluOpType.is_equal,
                    fill=0.0,
                    base=0,
                    pattern=[[-1, 128]],  # iota = row - col == 0 -> keep
                    channel_multiplier=1,
                )
                nc.vector.tensor_copy(ident16[:], ident[:])
                # 0/1 triangle: keep (col >= row), zero elsewhere.
                nc.gpsimd.memset(msk_t[:], 1.0)
                nc.gpsimd.affine_select(
                    out=msk_t[:],
                    in_=msk_t[:],
                    compare_op=mybir.AluOpType.is_ge,
                    fill=0.0,
                    base=0,
                    pattern=[[1, TT]],  # iota = col - row >= 0 -> keep
                    channel_multiplier=-1,
                )

            built = {"done": False}
            if not trap:
                # full-width per-g masks: keep col >= row + 128g
                msk4_t = cpool.tile([128, 4, TT], F32, tag="msk4")
                nc.gpsimd.memset(msk4_t[:], 1.0)
                for g in range(4):
                    nc.gpsimd.affine_select(
                        out=msk4_t[:, g, :],
                        in_=msk4_t[:, g, :],
                        compare_op=mybir.AluOpType.is_ge,
                        fill=0.0,
                        base=-128 * g,
                        pattern=[[1, TT]],
                        channel_multiplier=-1,
                    )

            os_engine = {"dve": nc.vector, "act": None, "split": "split"}[os_eng]

            def body():
                qT = wpool.tile([128, 2, S], F32R, tag="qT")
                kT = wpool.tile([128, 2, S], F32R, tag="kT")
                v_nat = wpool.tile([128, 2, NKJ, 2, 65], PT_DT, tag="v_nat")

                xt_ts = {}

                def load_xt(tt, eng=nc.gpsimd, chunked=False):
                    t = xtpool.tile([128, 8, TT], F32R, tag="xt", name=f"xt{tt}")
                    if chunked:
                        for dc in range(8):
                            eng.dma_start(t[:, dc, :], xt_r[:, dc, ts(tt, TT)])
                    else:
                        eng.dma_start(t[:], xt_r[:, :, ts(tt, TT)])
                    xt_ts[tt] = t

                def attn(hp, qi, z2_tile, defer=None, weave=(), inject=None):
                    """Score/softmax/AV for head-pair hp over q block qi.

                    defer: emitted after the first 4 score/exp groups and
                    before their AVs (hides v-repack latency in the tt loop).
                    weave: filler thunks (QKV chunks of the next t block /
                    projection groups of the previous q block) emitted
                    BETWEEN each score/exp group and its AV so PE has
                    independent work while Act's exp drains, plus one after
                    the final AV to cover the reciprocal latency.
                    inject: the previous block's normalization closure,
                    emitted after this block's first score groups so its
                    reciprocal chain hides behind fresh PE work.
                    Returns this block's normalization closure (the caller
                    must run it, directly or via the next attn's inject).
                    """
                    n_kj = 4 * qi + 4
                    weave = list(weave)
                    wk_at = {}
                    if n_end_cfg == "adaptive":
                        ne = 2 if n_kj <= 8 else 1
                    else:
                        ne = n_end_cfg
                    n_end = min(len(weave), ne) if weave else 0
                    spread, tail_w = (
                        weave[: len(weave) - n_end],
                        weave[len(weave) - n_end :],
                    )
                    for wi, wfn in enumerate(spread):
                        wk_at.setdefault(wi * n_kj // len(spread), []).append(wfn)
                    zp = [
                        psZ.tile([65, TT], F32, tag="zp", name="zp0"),
                        psZ.tile([65, TT], F32, tag="zp", name="zp1"),
                    ]
                    pts = {}

                    def sc_exp_mask(kj):
                        g = kj - 4 * qi
                        w = TT if (g < 0 or not trap) else TT - 128 * g
                        c0 = TT - w
                        sc = psA.tile([128, 2, TT], F32, tag="big", name="sc")
                        for h in (0, 1):
                            nc.tensor.matmul(
                                sc[:, h, c0:TT],
                                kT[64 * h : 64 * h + 64, hp, ts(kj, 128)],
                                qT[64 * h : 64 * h + 64, hp, TT * qi + c0 : TT * qi + TT],
                                start=True,
                                stop=True,
                            )
                        pt = ptpool.tile([128, 2, TT], PT_DT, tag="pt", name="pt")
                        nc.scalar.activation(
                            pt[:, :, c0:TT], sc[:, :, c0:TT], Exp, scale=0.125
                        )
                        if g >= 0:
                            if msk_eng == "split":
                                mengs = (nc.vector, nc.gpsimd)
                            else:
                                mengs = (nc.vector,) * 2 if msk_eng == "dve" else (nc.gpsimd,) * 2
                            mop = msk_t[:, 0:w] if trap else msk4_t[:, g, :]
                            for h in (0, 1):
                                mengs[h].tensor_mul(
                                    pt[:, h, c0:TT], pt[:, h, c0:TT], mop
                                )
                        pts[kj] = (pt, c0)

                    def av(kj):
                        pt, c0 = pts.pop(kj)
                        for h in (0, 1):
                            nc.tensor.matmul(
                                zp[h][:, c0:TT],
                                v_nat[:, hp, kj, h, :],
                                pt[:, h, c0:TT],
                                start=(kj == 0),
                                stop=(kj == n_kj - 1),
                                skip_group_check=True,
                            )

                    if wmode == "tail":
                        tail_w = spread + tail_w
                        wk_at = {}

                    def run_weaves(kj):
                        for wfn in wk_at.get(kj, ()):
                            wfn()

                    if defer is not None:
                        for kj in range(4):
                            sc_exp_mask(kj)
                        defer()
                        if inject is not None:
                            inject()
                        for kj in range(4):
                            if wmode == "pre_av":
                                run_weaves(kj)
                                av(kj)
                            else:
                                av(kj)
                                run_weaves(kj)
                        for kj in range(4, n_kj):
                            sc_exp_mask(kj)
                            if wmode == "pre_av":
                                run_weaves(kj)
                                av(kj)
                            else:
                                av(kj)
                                run_weaves(kj)
                    else:
                        for kj in range(min(3, n_kj)):
                            sc_exp_mask(kj)
                        if inject is not None:
                            inject()
                        for kj in range(min(3, n_kj)):
                            if wmode == "pre_av":
                                run_weaves(kj)
                                av(kj)
                            else:
                                av(kj)
                                run_weaves(kj)
                        for kj in range(min(3, n_kj), n_kj):
                            sc_exp_mask(kj)
                            if wmode == "pre_av":
                                run_weaves(kj)
                                av(kj)
                            else:
                                av(kj)
                                run_weaves(kj)
                    for wfn in tail_w:
                        wfn()

                    def norm():
                        rb = psA.tile([128, 2, TT], F32, tag="big", name="rb")
                        for h in (1, 0):
                            rec = recpool.tile([65, TT], F32R, tag="rec", name="rec")
                            with nc.allow_low_precision(reason="fp32r softmax recip"):
                                nc.vector.reciprocal(rec[64:65, :], zp[h][64:65, :])
                            nc.tensor.matmul(
                                rb[0:64, h, :],
                                ones_t[64:65, 0:64].bitcast(F32R),
                                rec[64:65, :],
                                start=True,
                                stop=True,
                            )
                            rbs = rbspool.tile([64, TT], F32R, tag="rbs", name="rbs")
                            nc.vector.tensor_copy(rbs[:], rb[0:64, h, :])
                            if h == 0:
                                nc.vector.tensor_mul(
                                    z2_tile[0:64, :], zp[0][0:64, :], rbs[:]
                                )
                            else:
                                stg = stgpool.tile(
                                    [64, TT], F32R, tag="stg", name="stg"
                                )
                                nc.vector.tensor_mul(stg[:], zp[1][0:64, :], rbs[:])
                                nc.sync.dma_start(z2_tile[64:128, :], stg[:])

                    return norm

                def qkv_thunks(tt, interleave_qk=False):
                    """QKV projection for t block tt as three weavable
                    thunks (q, k, v) plus the deferred v-repack. With
                    interleave_qk (tt=0), q and k matmuls alternate per dc
                    chunk so PE consumption matches the xt DMA supply rate
                    during startup."""
                    state = {}

                    def chunk_qk():
                        xt_t = xt_ts[tt]
                        ps_q = psA.tile([128, 2, TT], F32, tag="big", name="ps_q")
                        ps_k = psA.tile([128, 2, TT], F32, tag="big", name="ps_k")
                        for dc in range(8):
                            for ps, w_t in ((ps_q, wq_t), (ps_k, wk_t)):
                                for et in (0, 1):
                                    nc.tensor.matmul(
                                        ps[:, et, :],
                                        w_t[:, dc, ts(et, 128)],
                                        xt_t[:, dc, :],
                                        start=(dc == 0),
                                        stop=(dc == 7),
                                    )
                        for dest, bias_t, ps in ((qT, bq_t, ps_q), (kT, bk_t, ps_k)):
                            for et in (0, 1):
                                nc.vector.tensor_scalar(
                                    dest[:, et, ts(tt, TT)],
                                    ps[:, et, :],
                                    bias_t[:, et : et + 1],
                                    None,
                                    mybir.AluOpType.add,
                                )

                    def chunk(which, w_t, bias_t, dest):
                        def fn():
                            xt_t = xt_ts[tt]
                            ps = psA.tile(
                                [128, 2, TT], F32, tag="big", name=f"ps_{which}"
                            )
                            for et in (0, 1):
                                for dc in range(8):
                                    nc.tensor.matmul(
                                        ps[:, et, :],
                                        w_t[:, dc, ts(et, 128)],
                                        xt_t[:, dc, :],
                                        start=(dc == 0),
                                        stop=(dc == 7),
                                    )
                            if which != "v":
                                for et in (0, 1):
                                    nc.vector.tensor_scalar(
                                        dest[:, et, ts(tt, TT)],
                                        ps[:, et, :],
                                        bias_t[:, et : et + 1],
                                        None,
                                        mybir.AluOpType.add,
                                    )
                            else:
                                del xt_ts[tt]
                                vsts = []
                                for et in (0, 1):
                                    vst = vstpool.tile(
                                        [128, TT], PT_DT, tag="vst", name="vst"
                                    )
                                    nc.scalar.activation(
                                        vst[:],
                                        ps[:, et, :],
                                        Identity,
                                        bias=bv_t[:, et : et + 1],
                                    )
                                    vsts.append(vst)
                                state["vsts"] = vsts

                        return fn

                    if interleave_qk:
                        thunks = [chunk_qk, chunk("v", wv_t, bv_t, None)]
                    else:
                        thunks = [
                            chunk("q", wq_t, bq_t, qT),
                            chunk("k", wk_t, bk_t, kT),
                            chunk("v", wv_t, bv_t, None),
                        ]

                    def vwork():
                        vneng = nc.vector
                        vsts = state["vsts"]
                        pst = psA.tile([128, 2, TT], PT_DT, tag="big", name="pst")
                        for et in (0, 1):
                            for j4 in range(4):
                                nc.tensor.transpose(
                                    pst[:, et, ts(j4, 128)],
                                    vsts[et][:, ts(j4, 128)],
                                    ident16[:],
                                )
                        for et in (0, 1):
                            for j4 in range(4):
                                vneng.tensor_copy(
                                    v_nat[:, et, 4 * tt + j4, :, 0:64],
                                    pst[:, et, ts(j4, 128)].rearrange(
                                        "p (h e) -> p h e", h=2
                                    ),
                                )

                    return thunks, vwork

                def proj_groups(qi, z2a, z2b, tail=False):
                    """One thunk per 128-token tile of the qi block. With
                    tail=True the out DMAs alternate SP/Act queues (Act is
                    idle at the end; serialized SP DMAs were the tail)."""
                    thunks = []
                    for j in range(4):
                        i = 4 * qi + j

                        def grp(j=j, i=i):
                            os_ = ospool.tile([128, D], F32, tag="os", name="os")
                            pp = psA.tile([128, 2, TT], F32, tag="big", name="pp")
                            for oh in (0, 1):
                                nc.tensor.matmul(
                                    pp[:, oh, :],
                                    z2a[:, ts(j, 128)],
                                    wp_t[:, 0, ts(oh, TT)],
                                    start=True,
                                    stop=False,
                                )
                                if tail and not no_tailsplit:
                                    # split z2b halves so the h0 rows (ready
                                    # before the h1 lane-shift DMA) project
                                    # without waiting on it
                                    nc.tensor.matmul(
                                        pp[:, oh, :],
                                        z2b[0:64, ts(j, 128)],
                                        wp_t[0:64, 1, ts(oh, TT)],
                                        start=False,
                                        stop=False,
                                    )
                                    nc.tensor.matmul(
                                        pp[:, oh, :],
                                        z2b[64:128, ts(j, 128)],
                                        wp_t[64:128, 1, ts(oh, TT)],
                                        start=False,
                                        stop=True,
                                    )
                                else:
                                    nc.tensor.matmul(
                                        pp[:, oh, :],
                                        z2b[:, ts(j, 128)],
                                        wp_t[:, 1, ts(oh, TT)],
                                        start=False,
                                        stop=True,
                                    )
                            if os_engine is None:
                                for oh in (0, 1):
                                    nc.scalar.activation(
                                        os_[:, ts(oh, TT)], pp[:, oh, :], Identity
                                    )
                            elif os_engine == "split":
                                nc.vector.tensor_copy(os_[:, 0:TT], pp[:, 0, :])
                                nc.scalar.activation(
                                    os_[:, TT:D], pp[:, 1, :], Identity
                                )
                            else:
                                os_engine.tensor_copy(os_[:], pp[:])
                            oeng = nc.scalar if act_dma else (nc.gpsimd if cfg.get("pool_odma", False) else nc.sync)
                            if tail and j == 3:
                                nc.sync.dma_start(out[ts(i, 128), 0:TT], os_[:, 0:TT])
                                oeng.dma_start(out[ts(i, 128), TT:D], os_[:, TT:D])
                            elif tail and j % 2 == 1:
                                oeng.dma_start(out[ts(i, 128), :], os_[:])
                            else:
                                nc.sync.dma_start(out[ts(i, 128), :], os_[:])

                        thunks.append(grp)
                    return thunks

                # ---- emission ----
                load_xt(0, chunked=True)
                load_xt(1, eng=(nc.scalar if act_dma else nc.gpsimd), chunked=True)
                if not built["done"]:
                    build_consts()
                    built["done"] = True
                nc.vector.tensor_copy(
                    v_nat[:, :, :, :, 64:65],
                    ones_t[:, 0:64].rearrange(
                        "p (g a h c) -> p g a h c", g=2, a=NKJ, h=2
                    ),
                )
                if weave_off:
                    for tt in range(2, NQI):
                        load_xt(tt)
                    vworks = {}
                    for tt in range(NQI):
                        thunks, vworks[tt] = qkv_thunks(tt)
                        for th in thunks:
                            th()
                        vworks[tt]()
                    z2as = []
                    for tt in range(NQI):
                        z2a = z2apool.tile([128, TT], F32R, tag="z2a", name="z2a")
                        attn(0, tt, z2a)()
                        z2as.append(z2a)
                    z2bs = []
                    for qi in range(NQI):
                        z2b = z2bpool.tile([128, TT], F32R, tag="z2b", name="z2b")
                        attn(1, qi, z2b)()
                        z2bs.append(z2b)
                        for grp in proj_groups(qi, z2as[qi], z2b):
                            grp()
                    return
                thunks0, vwork0 = qkv_thunks(0, interleave_qk=not no_ilqk)
                for th in thunks0:
                    th()
                vworks = {0: vwork0}
                pending = None
                z2as = []
                for tt in range(NQI):
                    w = []
                    if tt + 1 < NQI:
                        nxt, vworks[tt + 1] = qkv_thunks(tt + 1)
                        if hp0_seq:
                            for th in nxt:
                                th()
                            if tt + 2 < NQI:
                                load_xt(tt + 2, chunked=not xt_whole23)
                        else:
                            if tt + 2 < NQI:
                                w.append(
                                    lambda tt=tt: load_xt(
                                        tt + 2, chunked=not xt_whole23
                                    )
                                )
                            w.extend(nxt)
                    z2a = z2apool.tile([128, TT], F32R, tag="z2a", name="z2a")
                    if no_defer:
                        vworks[tt]()
                        norm = attn(0, tt, z2a, weave=w, inject=pending)
                    else:
                        norm = attn(
                            0, tt, z2a, defer=vworks[tt], weave=w, inject=pending
                        )
                    if use_inject:
                        pending = norm
                    else:
                        norm()
                    z2as.append(z2a)
                z2bs = []
                for qi in range(NQI):
                    z2b = z2bpool.tile([128, TT], F32R, tag="z2b", name="z2b")
                    grps = (
                        proj_groups(qi - 1, z2as[qi - 1], z2bs[qi - 1])
                        if qi >= 1
                        else []
                    )
                    if hp1_seq:
                        nw = 0
                    elif max_weave is None:
                        nw = len(grps)
                    else:
                        nw = min(max_weave, len(grps)) if qi == 1 else 0
                    norm = attn(1, qi, z2b, weave=grps[:nw], inject=pending)
                    if use_inject:
                        pending = norm
                    else:
                        norm()
                    for grp in grps[nw:]:
                        grp()
                    z2bs.append(z2b)
                if use_inject:
                    pending()
                for grp in proj_groups(
                    NQI - 1, z2as[NQI - 1], z2bs[NQI - 1], tail=True
                ):
                    grp()

            if reps == 1:
                body()
            else:
                engs = (
                    mybir.EngineType.PE,
                    mybir.EngineType.Activation,
                    mybir.EngineType.DVE,
                    mybir.EngineType.SP,
                )
                with tc.For_i(0, reps, 1, hint_engines=engs):
                    body()

    _legalize_multi_waits(nc)
    return nc


def _host_inputs(x, W_qkv, b_qkv, W_proj):
    """Full inputs -> list of per-core input dicts."""
    x = np.asarray(x, dtype=np.float32)
    W_qkv = np.asarray(W_qkv, dtype=np.float32)
    b_qkv = np.asarray(b_qkv, dtype=np.float32)
    W_proj = np.asarray(W_proj, dtype=np.float32)

    xts = [np.ascontiguousarray(x[b].T) for b in range(B)]

    in_maps = []
    for c in range(NCORES):
        b = c // 4
        hg = c % 4
        cols = slice(256 * hg, 256 * hg + 256)
        in_maps.append(
            {
                "xt": xts[b],
                "wq": np.ascontiguousarray(W_qkv[:, 0:1024][:, cols]).reshape(
                    8, 128, 256
                ),
                "wk": np.ascontiguousarray(W_qkv[:, 1024:2048][:, cols]).reshape(
                    8, 128, 256
                ),
                "wv": np.ascontiguousarray(W_qkv[:, 2048:3072][:, cols]).reshape(
                    8, 128, 256
                ),
                "bqkv": np.concatenate(
                    [
                        b_qkv[1024 * s : 1024 * s + 1024][cols].reshape(2, 128).T
                        for s in range(3)
                    ],
                    axis=1,
                ),
                "wp": np.ascontiguousarray(W_proj[cols, :]).reshape(2, 128, D),
            }
        )
    return in_maps


_module_cache = {}

BEST_CFG = {"no_tailsplit": True, "msk_eng": "pool", "pool_odma": True, "os_bufs": 3, "n_end": 1}


def _get_module(reps: int = 1):
    if reps not in _module_cache:
        _module_cache[reps] = build_module(reps, BEST_CFG)
    return _module_cache[reps]


def run_on_device(in_maps, reps: int = 1):
    from concourse.bass_utils import run_bass_kernel_spmd

    nc = _get_module(reps)
    return run_bass_kernel_spmd(nc, in_maps, core_ids=list(range(NCORES)), trace=False)


def kernel(x, W_qkv, b_qkv, W_proj, b_proj):
    in_maps = _host_inputs(x, W_qkv, b_qkv, W_proj)
    res = run_on_device(in_maps, reps=1)
    b_proj = np.asarray(b_proj, dtype=np.float32)
    out = np.empty((B, S, D), dtype=np.float32)
    for b in range(B):
        acc = res.results[4 * b]["out"].copy()
        for c in range(4 * b + 1, 4 * b + 4):
            acc += res.results[c]["out"]
        out[b] = acc + b_proj
    return out
